# revision 1
# baseline (speedup 1.0000x reference)
"""Trainium2 Bass kernel for nn_AttentionBlock (sparse bilinear attention).

Reference computation (N_NET=1, D=4, N_H=8, N_T=2048, N_IN=N_OUT=256):
    Omega[N,b,h,t,u] = r'[N,b,t,i] Q[N,h,i,j] r'[N,b,u,j]
    Omega *= tril(ones(T, T))                      # causal mask
    r[N,b,t,i] = Omega[N,b,h,t,u] E[N,h,i,j] r'[N,b,u,j]   # sums over h

Sharding across 8 NeuronCores: core c handles batch b = c//2 and the 4
heads [4*(c%2), 4*(c%2)+4). Each core produces the partial output for its
batch summed over its 4 heads; the host adds the two head-group partials.

Per-core device algorithm (everything transposed so contractions are on
the partition axis; fp32r matmuls = full PE rate):
    AT[h](j,t)  = sum_i Q[h](i,j) rT(i,t)          (j on partitions)
    V[h](u,i)   = sum_j rT(j,u) ET[h](j,i)         (u on partitions)
    S(u,t)      = sum_j rT(j,u) AT[h](j,t)         = Omega^T tile
    outT(i,t)  += sum_u V[h](u,i) S(u,t)           (PSUM-accumulated over
                                                    all heads & u-blocks)
Causality (keep u <= t): u-blocks entirely above the diagonal are skipped,
diagonal tiles are computed only on their nonzero column range with a
128x128 triangular mask applied to the crossing sub-block.
"""

import numpy as np

N_T = 2048
N_IN = 256
T_TILE = 512
TT = N_T // T_TILE  # 4 t-tiles
UB = N_T // 128     # 16 u-blocks
HL = 4              # heads per core
N_CORES = 8

_cache = {}


def _tri_mask():
    # mask[p, c] = 1 if c >= p  (keep u <= t on the diagonal sub-block)
    idx = np.arange(128)
    return (idx[None, :] >= idx[:, None]).astype(np.float32)


def _build_nc(repeat=1):
    import concourse.tile as tile
    import concourse.mybir as mybir
    from concourse import bacc

    F32 = mybir.dt.float32
    F32R = mybir.dt.float32r

    nc = bacc.Bacc("TRN2", target_bir_lowering=False, debug=False,
                   num_devices=N_CORES)
    rT_d = nc.dram_tensor("rT", (2, 128, N_T), F32R, kind="ExternalInput").ap()
    Q4_d = nc.dram_tensor("Q4", (HL, 2, 128, N_IN), F32R,
                          kind="ExternalInput").ap()
    ET4_d = nc.dram_tensor("ET4", (HL, 2, 128, N_IN), F32R,
                           kind="ExternalInput").ap()
    mask_d = nc.dram_tensor("mask", (128, 128), F32R,
                            kind="ExternalInput").ap()
    outT_d = nc.dram_tensor("outT", (2, 128, N_T), F32,
                            kind="ExternalOutput").ap()

    with tile.TileContext(nc) as tc:
        with (
            tc.tile_pool(name="const", bufs=1) as const,
            tc.tile_pool(name="spool", bufs=4) as spool,
            tc.tile_pool(name="opool", bufs=4) as opool,
            tc.tile_pool(name="psum", bufs=4, space="PSUM") as psum,
            tc.tile_pool(name="psout", bufs=4, space="PSUM") as psout,
        ):
            rT_sb = const.tile([128, 2, N_T], F32R)
            Q_sb = const.tile([128, HL, 2, N_IN], F32R)
            ET_sb = const.tile([128, HL, 2, N_IN], F32R)
            mask_sb = const.tile([128, 128], F32R)
            for ic in range(2):
                nc.sync.dma_start(out=rT_sb[:, ic, :], in_=rT_d[ic])
            for hl in range(HL):
                for ic in range(2):
                    nc.sync.dma_start(out=Q_sb[:, hl, ic, :], in_=Q4_d[hl, ic])
                    nc.sync.dma_start(out=ET_sb[:, hl, ic, :],
                                      in_=ET4_d[hl, ic])
            nc.sync.dma_start(out=mask_sb, in_=mask_d)

            AT = [const.tile([128, 2, N_T], F32R, name=f"AT{h}")
                  for h in range(HL)]
            V = [const.tile([128, UB, N_IN], F32R, name=f"V{h}")
                 for h in range(HL)]

            for _rep in range(repeat):
                # ---- Phase A+V: per-head AT and V ----
                for hl in range(HL):
                    for jc in range(2):
                        for tt in range(TT):
                            ts = slice(T_TILE * tt, T_TILE * (tt + 1))
                            ps_a = psum.tile([128, T_TILE], F32, tag="ps",
                                             name="ps_a")
                            for ic in range(2):
                                nc.tensor.matmul(
                                    ps_a,
                                    Q_sb[:, hl, ic, 128 * jc:128 * (jc + 1)],
                                    rT_sb[:, ic, ts],
                                    start=(ic == 0), stop=(ic == 1))
                            # psum -> SBUF with fp32r rounding
                            if (jc + tt) % 2 == 0:
                                nc.scalar.copy(AT[hl][:, jc, ts], ps_a)
                            else:
                                nc.vector.tensor_copy(AT[hl][:, jc, ts], ps_a)
                    for ub in range(UB):
                        us = slice(128 * ub, 128 * (ub + 1))
                        ps_v = psum.tile([128, N_IN], F32, tag="ps",
                                         name="ps_v")
                        for jc in range(2):
                            nc.tensor.matmul(
                                ps_v, rT_sb[:, jc, us], ET_sb[:, hl, jc, :],
                                start=(jc == 0), stop=(jc == 1))
                        if ub % 2 == 0:
                            nc.scalar.copy(V[hl][:, ub, :], ps_v)
                        else:
                            nc.vector.tensor_copy(V[hl][:, ub, :], ps_v)

                # ---- Main: S tiles + PSUM-accumulated output ----
                for tt in range(TT):
                    t0 = T_TILE * tt
                    po = [psout.tile([128, T_TILE], F32, tag="po",
                                     name=f"po{ic}") for ic in range(2)]
                    n_ub = 4 * tt + 4
                    full_copy_parity = 0
                    for hl in range(HL):
                        for ub in range(n_ub):
                            d = ub - 4 * tt  # >=0: diagonal sub-block index
                            lo = max(d, 0) * 128
                            ts_s = slice(t0 + lo, t0 + T_TILE)
                            us = slice(128 * ub, 128 * (ub + 1))
                            ps_s = psum.tile([128, T_TILE], F32, tag="ps",
                                             name="ps_s")
                            for jc in range(2):
                                nc.tensor.matmul(
                                    ps_s[:, lo:T_TILE],
                                    rT_sb[:, jc, us],
                                    AT[hl][:, jc, ts_s],
                                    start=(jc == 0), stop=(jc == 1))
                            s_sb = spool.tile([128, T_TILE], F32R, tag="s",
                                              name="s_sb")
                            if d >= 0:
                                nc.vector.tensor_mul(
                                    s_sb[:, lo:lo + 128],
                                    ps_s[:, lo:lo + 128], mask_sb)
                                if lo + 128 < T_TILE:
                                    nc.vector.tensor_copy(
                                        s_sb[:, lo + 128:T_TILE],
                                        ps_s[:, lo + 128:T_TILE])
                            else:
                                if full_copy_parity % 2 == 0:
                                    nc.scalar.copy(s_sb, ps_s)
                                else:
                                    nc.vector.tensor_copy(s_sb, ps_s)
                                full_copy_parity += 1
                            first = (hl == 0 and ub == 0)
                            last = (hl == HL - 1 and ub == n_ub - 1)
                            for ic in range(2):
                                nc.tensor.matmul(
                                    po[ic][:, lo:T_TILE],
                                    V[hl][:, ub, 128 * ic:128 * (ic + 1)],
                                    s_sb[:, lo:T_TILE],
                                    start=first, stop=last,
                                    skip_group_check=True)
                    for ic in range(2):
                        ot = opool.tile([128, T_TILE], F32, tag="ot",
                                        name="ot")
                        nc.scalar.copy(ot, po[ic])
                        nc.sync.dma_start(
                            out=outT_d[ic, :, t0:t0 + T_TILE], in_=ot)
    nc.compile()
    return nc


def _prep_in_maps(r_prime, E, Q):
    mask = _tri_mask()
    in_maps = []
    for c in range(N_CORES):
        b, hg = divmod(c, 2)
        heads = slice(4 * hg, 4 * hg + 4)
        rT = np.ascontiguousarray(r_prime[0, b].T).reshape(2, 128, N_T)
        Q4 = np.ascontiguousarray(Q[0, heads]).reshape(HL, 2, 128, N_IN)
        ET4 = np.ascontiguousarray(
            E[0, heads].transpose(0, 2, 1)).reshape(HL, 2, 128, N_IN)
        in_maps.append({"rT": rT.astype(np.float32),
                        "Q4": Q4.astype(np.float32),
                        "ET4": ET4.astype(np.float32),
                        "mask": mask})
    return in_maps


def kernel(r_prime, E, Q):
    from concourse import bass_utils

    if "nc" not in _cache:
        _cache["nc"] = _build_nc()
    nc = _cache["nc"]
    in_maps = _prep_in_maps(r_prime, E, Q)
    res = bass_utils.run_bass_kernel_spmd(nc, in_maps,
                                          core_ids=list(range(N_CORES)))
    out = np.zeros((1, 4, N_T, N_IN), dtype=np.float32)
    for b in range(4):
        acc = (res.results[2 * b]["outT"].reshape(N_IN, N_T)
               + res.results[2 * b + 1]["outT"].reshape(N_IN, N_T))
        out[0, b] = acc.T
    return out


# revision 13
# speedup vs baseline: 17465.0898x; 17465.0898x over previous
"""Trainium2 Bass kernel for nn_AttentionBlock (sparse bilinear attention).

Reference computation (N_NET=1, D=4, N_H=8, N_T=2048, N_IN=N_OUT=256):
    Omega[N,b,h,t,u] = r'[N,b,t,i] Q[N,h,i,j] r'[N,b,u,j]
    Omega *= tril(ones(T, T))                      # causal mask
    r[N,b,t,i] = Omega[N,b,h,t,u] E[N,h,i,j] r'[N,b,u,j]   # sums over h

Sharding across 8 NeuronCores: core c handles batch b = c//2 and the 4
heads [4*(c%2), 4*(c%2)+4). Each core produces the partial output for its
batch summed over its 4 heads; the host adds the two head-group partials.

Per-core device algorithm (contractions on the partition axis; float32r
matmuls run at full PE rate with ~2e-4 relative error):
    AT[h](j,t)  = sum_i Q[h](i,j) rT(i,t)          (j on partitions)
    V[h](u,i)   = sum_j rT(j,u) ET[h](j,i)         (u on partitions,
                                                    two heads per matmul)
    S(u,t)      = sum_j rT(j,u) AT[h](j,t)         = Omega^T tile
    outT(i,t)  += sum_u V[h](u,i) S(u,t)           (PSUM-accumulated over
                                                    all heads & u-blocks)
Causality (keep u <= t): u-blocks entirely above the diagonal are skipped,
diagonal tiles are computed only on their nonzero column range with a
128x128 triangular mask applied to the crossing sub-block.
"""

import numpy as np

N_T = 2048
N_IN = 256
T_TILE = 512
TT = N_T // T_TILE  # 4 t-tiles
UB = N_T // 128     # 16 u-blocks
HL = 4              # heads per core
N_CORES = 8

_cache = {}


def _tri_mask():
    # mask[p, c] = 1 if c >= p  (keep u <= t on the diagonal sub-block)
    idx = np.arange(128)
    return (idx[None, :] >= idx[:, None]).astype(np.float32)


def _build_nc(repeat=1, bf16=False):
    import concourse.tile as tile
    import concourse.mybir as mybir
    from concourse import bacc

    F32 = mybir.dt.float32
    F32R = mybir.dt.bfloat16 if bf16 else mybir.dt.float32r

    nc = bacc.Bacc("TRN2", target_bir_lowering=False, debug=False,
                   num_devices=N_CORES)
    rT_d = nc.dram_tensor("rT", (2, 128, N_T), F32R, kind="ExternalInput").ap()
    Q4_d = nc.dram_tensor("Q4", (HL, 2, 128, N_IN), F32R,
                          kind="ExternalInput").ap()
    ET4_d = nc.dram_tensor("ET4", (HL, 2, 128, N_IN), F32R,
                           kind="ExternalInput").ap()
    mask_d = nc.dram_tensor("mask", (128, 128), F32R,
                            kind="ExternalInput").ap()
    outT_d = nc.dram_tensor("outT", (2, 128, N_T), F32,
                            kind="ExternalOutput").ap()

    # running per-engine copy-cost estimates for greedy DVE/ACT balancing
    eng_load = {"v": 0.0, "s": 0.0}

    def copy_psum(out_ap, in_ap, n):
        dve = n / 0.96 + 150.0
        act = (n + 352.0) / 1.2
        if eng_load["v"] + dve <= eng_load["s"] + act:
            eng_load["v"] += dve
            nc.vector.tensor_copy(out_ap, in_ap)
        else:
            eng_load["s"] += act
            nc.scalar.copy(out_ap, in_ap)

    with tile.TileContext(nc) as tc:
        with (
            tc.tile_pool(name="const", bufs=1) as const,
            tc.tile_pool(name="spool", bufs=8) as spool,
            tc.tile_pool(name="opool", bufs=4) as opool,
            tc.tile_pool(name="psum", bufs=5, space="PSUM") as psum,
            tc.tile_pool(name="psout", bufs=3, space="PSUM") as psout,
        ):
            # --- inputs, finely tiled so compute can start ASAP ---
            mask_sb = const.tile([128, 128], F32R)
            Q_h = [const.tile([128, 2, N_IN], F32R, name=f"Qh{h}")
                   for h in range(HL)]
            rT_t = [[const.tile([128, T_TILE], F32R, name=f"rT{ic}_{tq}")
                     for tq in range(TT)] for ic in range(2)]
            ET_p = [const.tile([128, 2, 2, N_IN], F32R, name=f"ETp{p}")
                    for p in range(2)]
            for ic in range(2):
                nc.sync.dma_start(out=Q_h[0][:, ic, :], in_=Q4_d[0, ic])
            for tq in range(TT):
                for ic in range(2):
                    nc.sync.dma_start(
                        out=rT_t[ic][tq],
                        in_=rT_d[ic, :, T_TILE * tq:T_TILE * (tq + 1)])
            for hl in range(1, HL):
                for ic in range(2):
                    nc.sync.dma_start(out=Q_h[hl][:, ic, :], in_=Q4_d[hl, ic])
            for p2 in range(2):
                for jc in range(2):
                    for h2 in range(2):
                        nc.sync.dma_start(out=ET_p[p2][:, jc, h2, :],
                                          in_=ET4_d[2 * p2 + h2, jc])
            nc.sync.dma_start(out=mask_sb, in_=mask_d)

            # u-block ub lives in rT tile [ub // 4], columns 128*(ub % 4)
            def rT_ub(jc, ub):
                c0 = 128 * (ub % 4)
                return rT_t[jc][ub // 4][:, c0:c0 + 128]

            AT = [[[const.tile([128, T_TILE], F32R, name=f"AT{h}_{j}_{t}")
                    for t in range(TT)] for j in range(2)] for h in range(HL)]
            # V pair tiles: [p2][ub] -> (128, 2 heads, 256)
            Vp = [[const.tile([128, 2, N_IN], F32R, name=f"V{p}_{u}")
                   for u in range(UB)] for p in range(2)]

            def emit_a(hl, tts):
                for tt in tts:
                    for jc in range(2):
                        ps_a = psum.tile([128, T_TILE], F32, tag="ps",
                                         name="ps_a")
                        for ic in range(2):
                            nc.tensor.matmul(
                                ps_a,
                                Q_h[hl][:, ic, 128 * jc:128 * (jc + 1)],
                                rT_t[ic][tt],
                                start=(ic == 0), stop=(ic == 1))
                        copy_psum(AT[hl][jc][tt], ps_a, T_TILE)

            def emit_v(p2, ubs):
                for ub in ubs:
                    ps_v = psum.tile([128, 2, N_IN], F32, tag="ps",
                                     name="ps_v")
                    for jc in range(2):
                        nc.tensor.matmul(
                            ps_v, rT_ub(jc, ub), ET_p[p2][:, jc, :, :],
                            start=(jc == 0), stop=(jc == 1))
                    copy_psum(Vp[p2][ub], ps_v, 2 * N_IN)

            def body():
                # ---- Phase A (AT per head), then Phase V ----
                for hl in range(HL):
                    emit_a(hl, range(TT))
                for p2 in range(2):
                    emit_v(p2, range(UB))

                # ---- Main: S tiles + PSUM-accumulated output ----
                for tt in range(TT):
                    t0 = T_TILE * tt
                    po = [psout.tile([128, T_TILE], F32, tag="po",
                                     name=f"po{ic}") for ic in range(2)]
                    n_ub = 4 * tt + 4
                    for hl in range(HL):
                        for ub in range(n_ub):
                            d = ub - 4 * tt  # >=0: diagonal sub-block index
                            lo = max(d, 0) * 128
                            width = T_TILE - lo
                            ps_s = psum.tile([128, T_TILE], F32, tag="ps",
                                             name="ps_s")
                            for jc in range(2):
                                nc.tensor.matmul(
                                    ps_s[:, lo:T_TILE],
                                    rT_ub(jc, ub),
                                    AT[hl][jc][tt][:, lo:T_TILE],
                                    start=(jc == 0), stop=(jc == 1))
                            s_sb = spool.tile([128, T_TILE], F32R, tag="s",
                                              name="s_sb")
                            if d >= 0:
                                nc.vector.tensor_mul(
                                    s_sb[:, lo:lo + 128],
                                    ps_s[:, lo:lo + 128], mask_sb)
                                eng_load["v"] += 128 / 0.96 + 150.0
                                if width > 128:
                                    nc.vector.tensor_copy(
                                        s_sb[:, lo + 128:T_TILE],
                                        ps_s[:, lo + 128:T_TILE])
                                    eng_load["v"] += (width - 128) / 0.96 + 150.0
                            else:
                                copy_psum(s_sb, ps_s, T_TILE)
                            first = (hl == 0 and ub == 0)
                            last = (hl == HL - 1 and ub == n_ub - 1)
                            for ic in range(2):
                                nc.tensor.matmul(
                                    po[ic][:, lo:T_TILE],
                                    Vp[hl // 2][ub][:, hl % 2,
                                                    128 * ic:128 * (ic + 1)],
                                    s_sb[:, lo:T_TILE],
                                    start=first, stop=last,
                                    skip_group_check=True)
                    for ic in range(2):
                        ot = opool.tile([128, T_TILE], F32, tag="ot",
                                        name="ot")
                        if ic == 0:
                            nc.vector.tensor_copy(ot, po[ic])
                        else:
                            nc.scalar.copy(ot, po[ic])
                        nc.sync.dma_start(
                            out=outT_d[ic, :, t0:t0 + T_TILE], in_=ot)

            if repeat == 1:
                body()
            elif repeat < 0:  # unrolled repeat (timing experiments)
                for _ in range(-repeat):
                    body()
            else:
                with tc.For_i(0, repeat, 1):
                    body()
    nc.compile()
    return nc


def _prep_in_maps(r_prime, E, Q, bf16=False):
    if bf16:
        import ml_dtypes
        cast_dt = ml_dtypes.bfloat16
    else:
        cast_dt = np.float32
    mask = _tri_mask()
    in_maps = []
    for c in range(N_CORES):
        b, hg = divmod(c, 2)
        heads = slice(4 * hg, 4 * hg + 4)
        rT = np.ascontiguousarray(r_prime[0, b].T).reshape(2, 128, N_T)
        Q4 = np.ascontiguousarray(Q[0, heads]).reshape(HL, 2, 128, N_IN)
        ET4 = np.ascontiguousarray(
            E[0, heads].transpose(0, 2, 1)).reshape(HL, 2, 128, N_IN)
        in_maps.append({"rT": rT.astype(cast_dt),
                        "Q4": Q4.astype(cast_dt),
                        "ET4": ET4.astype(cast_dt),
                        "mask": mask.astype(cast_dt)})
    return in_maps


def kernel(r_prime, E, Q):
    from concourse import bass_utils

    if "nc" not in _cache:
        _cache["nc"] = _build_nc()
    nc = _cache["nc"]
    in_maps = _prep_in_maps(r_prime, E, Q)
    res = bass_utils.run_bass_kernel_spmd(nc, in_maps,
                                          core_ids=list(range(N_CORES)))
    out = np.zeros((1, 4, N_T, N_IN), dtype=np.float32)
    for b in range(4):
        acc = (res.results[2 * b]["outT"].reshape(N_IN, N_T)
               + res.results[2 * b + 1]["outT"].reshape(N_IN, N_T))
        out[0, b] = acc.T
    return out
